# revision 1
# baseline (speedup 1.0000x reference)
# Trainium2 Bass kernel for nn_CalculateAttention_7722351198463
#
# reference computes, per (batch, head):
#   scores = (Qx @ Kx^T + Qy @ Ky^T) * 0.5 / sqrt(D)
#   attn   = softmax(scores, axis=-1)
#   out1   = attn @ Vx ; out2 = attn @ Vy
#
# Sharding: B*H = 64 heads, 8 heads per core across 8 NeuronCores (no comms).
#
# Device-side design (per core, 8 heads):
#  * Host pre-transposes Q/K into QT/KT = [d=128, s=1024] per head where the
#    x-stream occupies partitions 0:64 and the y-stream 64:128.  One matmul
#    with contraction 128 then computes Qx@Kx^T + Qy@Ky^T in a single pass
#    (full PE array utilization), directly in transposed [t, s] layout.
#  * exp() on ScalarE (scale folded into the activation), output bf16.
#  * V is packed host-side as VC = [t, 132] = [Vx | Vy | ones | pad] so that
#    one accumulating matmul chain computes [out1_raw | out2_raw | sumexp]
#    for each s-tile; softmax normalization is applied at the end as a
#    per-partition scalar multiply by 1/sumexp on VectorE.
#  * No transposes anywhere on the device; all matmuls are bf16 (1 cyc/row).
#  * Software-pipelined by one head (ACT exp of head h overlaps PE's PV of
#    head h-1); the last head's PV runs 8 interleaved PSUM accumulation
#    groups in j-outer order so it chases exp availability.
import numpy as np
import ml_dtypes

B, H, S, D = 4, 16, 1024, 64
N_CORES = 8
HEADS = B * H              # 64
HPC = HEADS // N_CORES     # heads per core = 8
ST = S // 128              # s/t tiles per head = 8
SCALE = 0.5 / 8.0          # 0.5 / sqrt(D)
VCW = 132                  # packed V width: 64 + 64 + 1 (ones) + 3 pad
INW = 2 * S + ST * VCW     # combined input row width = 3104

TRACE = False
TRACE_KW: dict = {}
LAST_RESULTS = None

_NC = None


def _build_bass():
    import concourse.mybir as mybir
    import concourse.tile as tile
    from concourse import bacc
    from concourse.tile import add_dep_helper

    f32 = mybir.dt.float32
    DT = mybir.dt.bfloat16
    EXP = mybir.ActivationFunctionType.Exp

    nc = bacc.Bacc("TRN2", target_bir_lowering=False, enable_partition_id=False)
    IN = nc.dram_tensor("inp", [HPC, 128, INW], DT, kind="ExternalInput")
    OC = nc.dram_tensor("oc", [HPC, 128, ST, VCW], f32, kind="ExternalOutput")

    with tile.TileContext(nc) as tc:
        with (
            tc.tile_pool(name="io", bufs=4) as io_pool,
            tc.tile_pool(name="exp", bufs=2) as exp_pool,
            tc.tile_pool(name="outs", bufs=2) as out_pool,
            tc.tile_pool(name="stat", bufs=8) as stat_pool,
            tc.tile_pool(name="spsum", bufs=2, space="PSUM") as s_psum,
            tc.tile_pool(name="opsum", bufs=4, space="PSUM") as o_psum,
        ):
            # Warm the ACT exp table during the DMA ramp so the ~2.7us
            # table-load is off the critical path.
            warm = stat_pool.tile([128, 1], f32, tag="warm")
            nc.gpsimd.memset(warm[:], 0.0)
            nc.scalar.activation(warm[:], warm[:], EXP)

            ins = [None] * HPC
            exps = [None] * HPC
            load_dmas = {}

            def emit_load(h):
                it = io_pool.tile([128, INW], DT, tag="in", name=f"in_{h}")
                # three DMAs per head -> three parallel DMA queues; head 0's
                # kt issues from the (still idle) scalar HWDGE queue so qt+kt
                # transfers start concurrently.
                kt_eng = nc.scalar if h == 0 else nc.sync
                d_qt = nc.sync.dma_start(it[:, 0:S], IN[h][:, 0:S])
                d_kt = kt_eng.dma_start(it[:, S:2 * S], IN[h][:, S:2 * S])
                nc.sync.dma_start(it[:, 2 * S:], IN[h][:, 2 * S:])
                load_dmas[h] = (d_qt, d_kt)
                if h == 1:
                    # Keep head 0's critical qt/kt transfers at full HBM
                    # bandwidth: head 1's loads start only once they land.
                    add_dep_helper(d_qt.ins, load_dmas[0][1].ins, sync=True,
                                   reason="stagger ramp DMA")
                ins[h] = it

            def emit_qk(h):
                it = ins[h]
                qt = it[:, 0:S]
                kt = it[:, S:2 * S]
                ex = exp_pool.tile([128, ST, S], DT, tag="exp")
                for j in range(ST):
                    # scoresT tile for t-tile j: [t=128, s=1024] (2 psum banks)
                    sps = s_psum.tile([128, S], f32, tag="scores")
                    for c in range(2):
                        nc.tensor.matmul(
                            sps[:, c * 512:(c + 1) * 512],
                            kt[:, j * 128:(j + 1) * 128],
                            qt[:, c * 512:(c + 1) * 512],
                            start=True, stop=True,
                        )
                    nc.scalar.activation(ex[:, j, :], sps[:], EXP, scale=SCALE)
                exps[h] = ex

            def emit_pv(h, chase=False):
                ex = exps[h]
                it = ins[h]

                def vc_j(j):
                    off = 2 * S + j * VCW
                    return it[:, off:off + 129]

                outt = out_pool.tile([128, ST, VCW], f32, tag="out")
                if chase:
                    # Last head: 8 interleaved accumulation groups, j-outer, so
                    # PV advances as each exp(j) lands.  Groups 4-7 live in two
                    # recycled scores-pool PSUM tiles (one group per bank).
                    psA = s_psum.tile([128, S], f32, tag="scores", name="chaseA")
                    psB = s_psum.tile([128, S], f32, tag="scores", name="chaseB")
                    opst = [
                        o_psum.tile([128, VCW], f32, tag="ops", name=f"ops_c{q}")
                        for q in range(4)
                    ] + [psA[:, 0:VCW], psA[:, 512:512 + VCW],
                         psB[:, 0:VCW], psB[:, 512:512 + VCW]]
                    for j in range(ST):
                        vj = vc_j(j)
                        for q in range(8):
                            nc.tensor.matmul(
                                opst[q][:, :129],
                                ex[:, j, q * 128:(q + 1) * 128],
                                vj,
                                start=(j == 0), stop=(j == ST - 1),
                            )
                    # two independent parallel evacuation chains (no
                    # cross-engine alternation): ACT takes groups 0-3,
                    # DVE takes 4-7
                    for q in range(4):
                        nc.scalar.copy(outt[:, q, :], opst[q][:])
                        if q % 2 == 1:
                            nc.sync.dma_start(
                                OC[h][:, q - 1:q + 1, :], outt[:, q - 1:q + 1, :])
                    for q in range(4, 8):
                        nc.vector.tensor_copy(outt[:, q, :], opst[q][:])
                        if q % 2 == 1:
                            nc.sync.dma_start(
                                OC[h][:, q - 1:q + 1, :], outt[:, q - 1:q + 1, :])
                else:
                    for half in range(2):
                        i0 = half * 4
                        opst = [
                            o_psum.tile([128, VCW], f32, tag="ops",
                                        name=f"ops_{half}_{q}")
                            for q in range(4)
                        ]
                        for q in range(4):
                            i = i0 + q
                            for j in range(ST):
                                nc.tensor.matmul(
                                    opst[q][:, :129],
                                    ex[:, j, i * 128:(i + 1) * 128],
                                    vc_j(j),
                                    start=(j == 0), stop=(j == ST - 1),
                                )
                        for q in range(4):
                            i = i0 + q
                            nc.vector.tensor_copy(outt[:, i, :], opst[q][:])
                        nc.sync.dma_start(
                            OC[h][:, i0:i0 + 4, :], outt[:, i0:i0 + 4, :])
                ins[h] = None
                exps[h] = None

            # Software-pipelined by one head: loads prefetch one head ahead
            # (deeper prefetch starves head 0's DMA bandwidth); ACT(exp) of
            # head h overlaps PE's PV of head h-1.
            emit_load(0)
            for h in range(HPC):
                if h + 1 < HPC:
                    emit_load(h + 1)
                emit_qk(h)
                if h >= 1:
                    emit_pv(h - 1)
            emit_pv(HPC - 1, chase=True)

    nc.compile()
    return nc


def _get_nc():
    global _NC
    if _NC is None:
        _NC = _build_bass()
    return _NC


def kernel(Qx, Kx, Vx, Qy, Ky, Vy):
    global LAST_RESULTS
    bf = ml_dtypes.bfloat16
    Qx, Kx, Vx, Qy, Ky, Vy = (
        np.asarray(t, dtype=np.float32) for t in (Qx, Kx, Vx, Qy, Ky, Vy)
    )

    qx = Qx.reshape(HEADS, S, D)
    qy = Qy.reshape(HEADS, S, D)
    kx = Kx.reshape(HEADS, S, D)
    ky = Ky.reshape(HEADS, S, D)
    vx = Vx.reshape(HEADS, S, D)
    vy = Vy.reshape(HEADS, S, D)

    # Combined per-head input block: [head, p=128, INW] where
    #   [:, 0:S]        = QT (x stream on partitions 0:64, y on 64:128)
    #   [:, S:2S]       = KT (same partition split)
    #   [:, 2S + j*VCW + c] = VC: kv position t = j*128+p; c in [Vx|Vy|1|pad]
    IN = np.zeros((HEADS, 128, INW), np.float32)
    IN[:, :D, 0:S] = qx.transpose(0, 2, 1)
    IN[:, D:, 0:S] = qy.transpose(0, 2, 1)
    IN[:, :D, S:2 * S] = kx.transpose(0, 2, 1)
    IN[:, D:, S:2 * S] = ky.transpose(0, 2, 1)
    vc = IN[:, :, 2 * S:].reshape(HEADS, 128, ST, VCW)
    vc[..., :D] = vx.reshape(HEADS, ST, 128, D).transpose(0, 2, 1, 3)
    vc[..., D:2 * D] = vy.reshape(HEADS, ST, 128, D).transpose(0, 2, 1, 3)
    vc[..., 2 * D] = 1.0

    in_maps = []
    for c in range(N_CORES):
        sl = slice(c * HPC, (c + 1) * HPC)
        in_maps.append({"inp": IN[sl].astype(bf)})

    from concourse.bass_utils import run_bass_kernel_spmd

    nc = _get_nc()
    res = run_bass_kernel_spmd(
        nc, in_maps, core_ids=list(range(N_CORES)), trace=TRACE, **TRACE_KW
    )
    LAST_RESULTS = res

    # oc: per core [HPC, p=128, i=ST, VCW]; cols 0:64 out1_raw, 64:128
    # out2_raw, col 128 sumexp -- softmax normalization happens here on host.
    oc = np.concatenate([r["oc"] for r in res.results], axis=0)
    oc = oc.transpose(0, 2, 1, 3).reshape(B, H, S, VCW)
    z = oc[..., 2 * D:2 * D + 1]
    out1 = np.ascontiguousarray(oc[..., :D] / z)
    out2 = np.ascontiguousarray(oc[..., D:2 * D] / z)
    return out1, out2



# revision 4
# speedup vs baseline: 1.0735x; 1.0735x over previous
# Trainium2 Bass kernel for nn_CalculateAttention_7722351198463
#
# reference computes, per (batch, head):
#   scores = (Qx @ Kx^T + Qy @ Ky^T) * 0.5 / sqrt(D)
#   attn   = softmax(scores, axis=-1)
#   out1   = attn @ Vx ; out2 = attn @ Vy
#
# Sharding: B*H = 64 heads, 8 heads per core across 8 NeuronCores (no comms).
#
# Device-side design (per core, 8 heads):
#  * Host pre-transposes Q/K into QT/KT = [d=128, s=1024] per head where the
#    x-stream occupies partitions 0:64 and the y-stream 64:128.  One matmul
#    with contraction 128 then computes Qx@Kx^T + Qy@Ky^T in a single pass
#    (full PE array utilization), directly in transposed [t, s] layout.
#    The softmax scale is folded into Q on the host (1/16 is exact in bf16).
#  * exp() on ScalarE, output bf16.  The ACT engine is the bottleneck
#    (~1.0 ns/elem + ~300 ns/instruction fixed cost), so exp instructions
#    are widened: per head the 8192 score columns are processed as
#    [2048, 1024, 2048, 1024, 2048]-wide activations using two PSUM score
#    buffers A=[128,2048] (4 banks) and B=[128,1024] (2 banks).
#  * V is packed host-side as VC = [t, 132] = [Vx | Vy | ones | pad] so that
#    one accumulating matmul chain computes [out1_raw | out2_raw | sumexp]
#    for each s-chunk; normalization happens on the host.
#  * PV accumulation groups are packed two per PSUM bank (offsets 0 and 132
#    within a [128, 264] tile) so 4 live groups need only 2 banks; PSUM
#    budget is exactly 4 (A) + 2 (B) + 2 (PV) = 8 banks.
#  * Software-pipelined by one head (ACT exp of head h overlaps PE's PV of
#    head h-1); the last head's PV chases exp availability: groups 0-3 run
#    j-outer in the PV banks as chunks land, groups 4-7 run j-inner in the
#    freed A banks after the final exp.
import numpy as np
import ml_dtypes

B, H, S, D = 4, 16, 1024, 64
N_CORES = 8
HEADS = B * H              # 64
HPC = HEADS // N_CORES     # heads per core = 8
ST = S // 128              # s/t tiles per head = 8
SCALE = 0.5 / 8.0          # 0.5 / sqrt(D), folded into Q on the host
VCW = 132                  # packed V width: 64 + 64 + 1 (ones) + 3 pad
INW = 2 * S + ST * VCW     # combined input row width = 3104
OW = ST * VCW              # flat output row width = 1056

# exp chunk schedule per head: (start col, width, which score buffer)
CHUNKS = [(0, 2048, 0), (2048, 1024, 1), (3072, 2048, 0),
          (5120, 1024, 1), (6144, 2048, 0)]

TRACE = False
TRACE_KW: dict = {}
LAST_RESULTS = None

_NC = None


def _build_bass():
    import concourse.mybir as mybir
    import concourse.tile as tile
    from concourse import bacc
    from concourse.tile import add_dep_helper

    f32 = mybir.dt.float32
    DT = mybir.dt.bfloat16
    EXP = mybir.ActivationFunctionType.Exp

    nc = bacc.Bacc("TRN2", target_bir_lowering=False, enable_partition_id=False)
    IN = nc.dram_tensor("inp", [HPC, 128, INW], DT, kind="ExternalInput")
    OC = nc.dram_tensor("oc", [HPC, 128, OW], f32, kind="ExternalOutput")

    with tile.TileContext(nc) as tc:
        with (
            tc.tile_pool(name="io", bufs=4) as io_pool,
            tc.tile_pool(name="exp", bufs=2) as exp_pool,
            tc.tile_pool(name="outs", bufs=2) as out_pool,
            tc.tile_pool(name="stat", bufs=8) as stat_pool,
            tc.tile_pool(name="psA", bufs=1, space="PSUM") as psA_pool,
            tc.tile_pool(name="psB", bufs=1, space="PSUM") as psB_pool,
            tc.tile_pool(name="opsum", bufs=2, space="PSUM") as o_psum,
        ):
            # Warm the ACT exp table during the DMA ramp so the ~2.7us
            # table-load is off the critical path.
            warm = stat_pool.tile([128, 1], f32, tag="warm")
            nc.gpsimd.memset(warm[:], 0.0)
            nc.scalar.activation(warm[:], warm[:], EXP)

            ins = [None] * HPC
            exps = [None] * HPC
            load_dmas = {}

            def emit_load(h):
                it = io_pool.tile([128, INW], DT, tag="in", name=f"in_{h}")
                # three DMAs per head -> three parallel DMA queues; head 0's
                # kt issues from the (still idle) scalar HWDGE queue so qt+kt
                # transfers start concurrently.
                kt_eng = nc.scalar if h == 0 else nc.sync
                d_qt = nc.sync.dma_start(it[:, 0:S], IN[h][:, 0:S])
                d_kt = kt_eng.dma_start(it[:, S:2 * S], IN[h][:, S:2 * S])
                nc.sync.dma_start(it[:, 2 * S:], IN[h][:, 2 * S:])
                load_dmas[h] = (d_qt, d_kt)
                if h == 1:
                    # Keep head 0's critical qt/kt transfers at full HBM
                    # bandwidth: head 1's loads start only once they land.
                    add_dep_helper(d_qt.ins, load_dmas[0][1].ins, sync=True,
                                   reason="stagger ramp DMA")
                ins[h] = it

            def emit_qk(h):
                it = ins[h]
                qt = it[:, 0:S]
                kt = it[:, S:2 * S]
                ex = exp_pool.tile([128, ST * S], DT, tag="exp")
                for st, w, which in CHUNKS:
                    sps = (psA_pool if which == 0 else psB_pool).tile(
                        [128, 2048 if which == 0 else 1024], f32, tag="sc")
                    for k in range(w // 512):
                        col = st + k * 512
                        j = col // S
                        nc.tensor.matmul(
                            sps[:, k * 512:(k + 1) * 512],
                            kt[:, j * 128:(j + 1) * 128],
                            qt[:, (col % S):(col % S) + 512],
                            start=True, stop=True,
                        )
                    nc.scalar.activation(ex[:, st:st + w], sps[:, 0:w], EXP)
                exps[h] = ex

            def vc_j(it, j):
                off = 2 * S + j * VCW
                return it[:, off:off + 129]

            def emit_pv(h):
                ex = exps[h]
                it = ins[h]
                outt = out_pool.tile([128, OW], f32, tag="out")
                for half in range(2):
                    tls = [o_psum.tile([128, 264], f32, tag="ops",
                                       name=f"ops_{h}_{half}_{t}")
                           for t in range(2)]
                    for g in range(4):
                        q = half * 4 + g
                        dst = tls[g // 2][:, (g % 2) * 132:(g % 2) * 132 + 129]
                        for j in range(ST):
                            nc.tensor.matmul(
                                dst,
                                ex[:, j * S + q * 128:j * S + (q + 1) * 128],
                                vc_j(it, j),
                                start=(j == 0), stop=(j == ST - 1),
                            )
                    for t in range(2):
                        o0 = (half * 4 + t * 2) * VCW
                        nc.vector.tensor_copy(outt[:, o0:o0 + 264], tls[t][:])
                    nc.sync.dma_start(
                        OC[h][:, half * 4 * VCW:(half + 1) * 4 * VCW],
                        outt[:, half * 4 * VCW:(half + 1) * 4 * VCW])
                ins[h] = None
                exps[h] = None

            def emit_pv_chase(h, sA):
                # Last head: no next head's exp to overlap with.  Only the
                # two PV banks are free while exp still runs, and a PSUM bank
                # cannot hold two simultaneously in-flight accumulation
                # groups, so groups 0-1 chase exp availability j-outer (one
                # group per PV bank); groups 2-7 run j-inner after the final
                # exp chunk, in the freed A score banks + recycled PV banks.
                ex = exps[h]
                it = ins[h]
                outt = out_pool.tile([128, OW], f32, tag="out")
                tls = [o_psum.tile([128, 264], f32, tag="ops",
                                   name=f"ops_c{t}") for t in range(2)]
                for j in range(ST):
                    vj = vc_j(it, j)
                    for q in range(2):
                        nc.tensor.matmul(
                            tls[q][:, 0:129],
                            ex[:, j * S + q * 128:j * S + (q + 1) * 128],
                            vj,
                            start=(j == 0), stop=(j == ST - 1),
                        )
                for t in range(2):
                    nc.vector.tensor_copy(outt[:, t * VCW:t * VCW + 132],
                                          tls[t][:, 0:132])
                nc.sync.dma_start(OC[h][:, 0:2 * VCW], outt[:, 0:2 * VCW])
                # tail: six j-inner chains pipelined on the PE
                tls2 = [o_psum.tile([128, 264], f32, tag="ops",
                                    name=f"ops_t{t}") for t in range(2)]
                dsts = [tls2[0][:, 0:129], tls2[1][:, 0:129]] + [
                    sA[:, k * 512:k * 512 + 129] for k in range(4)]
                for j in range(ST):
                    vj = vc_j(it, j)
                    for q in range(2, 8):
                        nc.tensor.matmul(
                            dsts[q - 2],
                            ex[:, j * S + q * 128:j * S + (q + 1) * 128],
                            vj,
                            start=(j == 0), stop=(j == ST - 1),
                        )
                for q in range(2, 4):
                    nc.vector.tensor_copy(outt[:, q * VCW:q * VCW + 132],
                                          tls2[q - 2][:, 0:132])
                for q in range(4, 8):
                    nc.scalar.copy(outt[:, q * VCW:q * VCW + 129],
                                   dsts[q - 2])
                nc.sync.dma_start(OC[h][:, 2 * VCW:], outt[:, 2 * VCW:])
                ins[h] = None
                exps[h] = None

            # Software-pipelined by one head: loads prefetch one head ahead
            # (deeper prefetch starves head 0's DMA bandwidth); ACT(exp) of
            # head h overlaps PE's PV of head h-1.
            emit_load(0)
            last_sA = None
            for h in range(HPC):
                if h + 1 < HPC:
                    emit_load(h + 1)
                emit_qk(h)
                if h == HPC - 1:
                    # the A tile of the last head, reused for chase groups
                    last_sA = psA_pool.tile([128, 2048], f32, tag="sc")
                if h >= 1:
                    emit_pv(h - 1)
            emit_pv_chase(HPC - 1, last_sA)

    nc.compile()
    return nc


def _get_nc():
    global _NC
    if _NC is None:
        _NC = _build_bass()
    return _NC


def kernel(Qx, Kx, Vx, Qy, Ky, Vy):
    global LAST_RESULTS
    bf = ml_dtypes.bfloat16
    Qx, Kx, Vx, Qy, Ky, Vy = (
        np.asarray(t, dtype=np.float32) for t in (Qx, Kx, Vx, Qy, Ky, Vy)
    )

    qx = Qx.reshape(HEADS, S, D)
    qy = Qy.reshape(HEADS, S, D)
    kx = Kx.reshape(HEADS, S, D)
    ky = Ky.reshape(HEADS, S, D)
    vx = Vx.reshape(HEADS, S, D)
    vy = Vy.reshape(HEADS, S, D)

    # Combined per-head input block: [head, p=128, INW] where
    #   [:, 0:S]        = QT * SCALE (x stream on partitions 0:64, y on 64:128)
    #   [:, S:2S]       = KT (same partition split)
    #   [:, 2S + j*VCW + c] = VC: kv position t = j*128+p; c in [Vx|Vy|1|pad]
    IN = np.zeros((HEADS, 128, INW), np.float32)
    IN[:, :D, 0:S] = qx.transpose(0, 2, 1) * SCALE
    IN[:, D:, 0:S] = qy.transpose(0, 2, 1) * SCALE
    IN[:, :D, S:2 * S] = kx.transpose(0, 2, 1)
    IN[:, D:, S:2 * S] = ky.transpose(0, 2, 1)
    vc = IN[:, :, 2 * S:].reshape(HEADS, 128, ST, VCW)
    vc[..., :D] = vx.reshape(HEADS, ST, 128, D).transpose(0, 2, 1, 3)
    vc[..., D:2 * D] = vy.reshape(HEADS, ST, 128, D).transpose(0, 2, 1, 3)
    vc[..., 2 * D] = 1.0

    in_maps = []
    for c in range(N_CORES):
        sl = slice(c * HPC, (c + 1) * HPC)
        in_maps.append({"inp": IN[sl].astype(bf)})

    from concourse.bass_utils import run_bass_kernel_spmd

    nc = _get_nc()
    res = run_bass_kernel_spmd(
        nc, in_maps, core_ids=list(range(N_CORES)), trace=TRACE, **TRACE_KW
    )
    LAST_RESULTS = res

    # oc: per core [HPC, p=128, ST*VCW]; per s-chunk q cols 0:64 out1_raw,
    # 64:128 out2_raw, col 128 sumexp -- softmax normalization here on host.
    oc = np.concatenate([r["oc"] for r in res.results], axis=0)
    oc = oc.reshape(HEADS, 128, ST, VCW).transpose(0, 2, 1, 3).reshape(B, H, S, VCW)
    z = oc[..., 2 * D:2 * D + 1]
    out1 = np.ascontiguousarray(oc[..., :D] / z)
    out2 = np.ascontiguousarray(oc[..., D:2 * D] / z)
    return out1, out2


# revision 5
# speedup vs baseline: 1.1891x; 1.1077x over previous
# Trainium2 Bass kernel for nn_CalculateAttention_7722351198463
#
# reference computes, per (batch, head):
#   scores = (Qx @ Kx^T + Qy @ Ky^T) * 0.5 / sqrt(D)
#   attn   = softmax(scores, axis=-1)
#   out1   = attn @ Vx ; out2 = attn @ Vy
#
# Sharding: B*H = 64 heads, 8 heads per core across 8 NeuronCores (no comms).
#
# Device-side design (per core, 8 heads):
#  * Host pre-transposes Q/K into QT/KT = [d=128, s=1024] per head where the
#    x-stream occupies partitions 0:64 and the y-stream 64:128.  One matmul
#    with contraction 128 then computes Qx@Kx^T + Qy@Ky^T in a single pass
#    (full PE array utilization), directly in transposed [t, s] layout.
#    The softmax scale is folded into Q on the host (1/16 is exact in bf16).
#  * exp() on ScalarE, output bf16.  The ACT engine is the bottleneck
#    (~1.0 ns/elem + ~300 ns/instruction fixed cost), so exp instructions
#    are widened: per head the 8192 score columns are processed as
#    [2048, 1024, 2048, 1024, 2048]-wide activations using two PSUM score
#    buffers A=[128,2048] (4 banks) and B=[128,1024] (2 banks).
#  * V is packed host-side as VC = [t, 132] = [Vx | Vy | ones | pad] so that
#    one accumulating matmul chain computes [out1_raw | out2_raw | sumexp]
#    for each s-chunk; normalization happens on the host.
#  * PV accumulation groups are packed two per PSUM bank (offsets 0 and 132
#    within a [128, 264] tile) so 4 live groups need only 2 banks; PSUM
#    budget is exactly 4 (A) + 2 (B) + 2 (PV) = 8 banks.
#  * Software-pipelined by one head (ACT exp of head h overlaps PE's PV of
#    head h-1); the last head's PV chases exp availability: groups 0-3 run
#    j-outer in the PV banks as chunks land, groups 4-7 run j-inner in the
#    freed A banks after the final exp.
import numpy as np
import ml_dtypes

B, H, S, D = 4, 16, 1024, 64
N_CORES = 8
HEADS = B * H              # 64
HPC = HEADS // N_CORES     # heads per core = 8
ST = S // 128              # s/t tiles per head = 8
SCALE = 0.5 / 8.0          # 0.5 / sqrt(D), folded into Q on the host
VCW = 132                  # packed V width: 64 + 64 + 1 (ones) + 3 pad
INW = 2 * S + ST * VCW     # combined input row width = 3104
OW = ST * VCW              # flat output row width = 1056

# exp chunk schedule per head: (start col, width, which score buffer).
# Strictly alternating A/B so no chunk ever waits on the previous chunk's
# buffer (including across heads): the score buffers are single-buffered.
CHUNKS = [(0, 2048, 0), (2048, 1024, 1), (3072, 2048, 0),
          (5120, 1024, 1), (6144, 1024, 0), (7168, 1024, 1)]

TRACE = False
TRACE_KW: dict = {}
LAST_RESULTS = None

_NC = None


def _build_bass():
    import concourse.mybir as mybir
    import concourse.tile as tile
    from concourse import bacc
    from concourse.tile import add_dep_helper

    f32 = mybir.dt.float32
    DT = mybir.dt.bfloat16
    EXP = mybir.ActivationFunctionType.Exp

    nc = bacc.Bacc("TRN2", target_bir_lowering=False, enable_partition_id=False)
    IN = nc.dram_tensor("inp", [HPC, 128, INW], DT, kind="ExternalInput")
    OC = nc.dram_tensor("oc", [HPC, 128, OW], f32, kind="ExternalOutput")

    with tile.TileContext(nc) as tc:
        with (
            tc.tile_pool(name="io", bufs=4) as io_pool,
            tc.tile_pool(name="exp", bufs=2) as exp_pool,
            tc.tile_pool(name="outs", bufs=2) as out_pool,
            tc.tile_pool(name="stat", bufs=8) as stat_pool,
            tc.tile_pool(name="psA", bufs=1, space="PSUM") as psA_pool,
            tc.tile_pool(name="psB", bufs=1, space="PSUM") as psB_pool,
            tc.tile_pool(name="opsum", bufs=2, space="PSUM") as o_psum,
        ):
            # Warm the ACT exp table during the DMA ramp so the ~2.7us
            # table-load is off the critical path.
            warm = stat_pool.tile([128, 1], f32, tag="warm")
            nc.gpsimd.memset(warm[:], 0.0)
            nc.scalar.activation(warm[:], warm[:], EXP)

            ins = [None] * HPC
            exps = [None] * HPC
            load_dmas = {}

            def emit_load(h):
                it = io_pool.tile([128, INW], DT, tag="in", name=f"in_{h}")
                # three DMAs per head -> three parallel DMA queues; head 0's
                # kt issues from the (still idle) scalar HWDGE queue so qt+kt
                # transfers start concurrently.
                kt_eng = nc.scalar if h == 0 else nc.sync
                d_qt = nc.sync.dma_start(it[:, 0:S], IN[h][:, 0:S])
                d_kt = kt_eng.dma_start(it[:, S:2 * S], IN[h][:, S:2 * S])
                nc.sync.dma_start(it[:, 2 * S:], IN[h][:, 2 * S:])
                load_dmas[h] = (d_qt, d_kt)
                if h == 1:
                    # Keep head 0's critical qt/kt transfers at full HBM
                    # bandwidth: head 1's loads start only once they land.
                    add_dep_helper(d_qt.ins, load_dmas[0][1].ins, sync=True,
                                   reason="stagger ramp DMA")
                ins[h] = it

            def emit_qk(h):
                it = ins[h]
                qt = it[:, 0:S]
                kt = it[:, S:2 * S]
                ex = exp_pool.tile([128, ST * S], DT, tag="exp")
                for st, w, which in CHUNKS:
                    sps = (psA_pool if which == 0 else psB_pool).tile(
                        [128, 2048 if which == 0 else 1024], f32, tag="sc")
                    for k in range(w // 512):
                        col = st + k * 512
                        j = col // S
                        nc.tensor.matmul(
                            sps[:, k * 512:(k + 1) * 512],
                            kt[:, j * 128:(j + 1) * 128],
                            qt[:, (col % S):(col % S) + 512],
                            start=True, stop=True,
                        )
                    nc.scalar.activation(ex[:, st:st + w], sps[:, 0:w], EXP)
                exps[h] = ex

            def vc_j(it, j):
                off = 2 * S + j * VCW
                return it[:, off:off + 129]

            def emit_pv(h):
                ex = exps[h]
                it = ins[h]
                outt = out_pool.tile([128, OW], f32, tag="out")
                for half in range(2):
                    tls = [o_psum.tile([128, 264], f32, tag="ops",
                                       name=f"ops_{h}_{half}_{t}")
                           for t in range(2)]
                    for g in range(4):
                        q = half * 4 + g
                        dst = tls[g // 2][:, (g % 2) * 132:(g % 2) * 132 + 129]
                        for j in range(ST):
                            nc.tensor.matmul(
                                dst,
                                ex[:, j * S + q * 128:j * S + (q + 1) * 128],
                                vc_j(it, j),
                                start=(j == 0), stop=(j == ST - 1),
                            )
                    for t in range(2):
                        o0 = (half * 4 + t * 2) * VCW
                        nc.vector.tensor_copy(outt[:, o0:o0 + 264], tls[t][:])
                    nc.sync.dma_start(
                        OC[h][:, half * 4 * VCW:(half + 1) * 4 * VCW],
                        outt[:, half * 4 * VCW:(half + 1) * 4 * VCW])
                ins[h] = None
                exps[h] = None

            def emit_pv_chase(h, sA):
                # Last head: no next head's exp to overlap with.  Only the
                # two PV banks are free while exp still runs, and a PSUM bank
                # cannot hold two simultaneously in-flight accumulation
                # groups, so groups 0-1 chase exp availability j-outer (one
                # group per PV bank); groups 2-7 run j-inner after the final
                # exp chunk, in the freed A score banks + recycled PV banks.
                ex = exps[h]
                it = ins[h]
                outt = out_pool.tile([128, OW], f32, tag="out")
                tls = [o_psum.tile([128, 264], f32, tag="ops",
                                   name=f"ops_c{t}") for t in range(2)]
                for j in range(ST):
                    vj = vc_j(it, j)
                    for q in range(2):
                        nc.tensor.matmul(
                            tls[q][:, 0:129],
                            ex[:, j * S + q * 128:j * S + (q + 1) * 128],
                            vj,
                            start=(j == 0), stop=(j == ST - 1),
                        )
                for t in range(2):
                    nc.vector.tensor_copy(outt[:, t * VCW:t * VCW + 132],
                                          tls[t][:, 0:132])
                nc.sync.dma_start(OC[h][:, 0:2 * VCW], outt[:, 0:2 * VCW])
                # tail: six j-inner chains pipelined on the PE
                tls2 = [o_psum.tile([128, 264], f32, tag="ops",
                                    name=f"ops_t{t}") for t in range(2)]
                dsts = [tls2[0][:, 0:129], tls2[1][:, 0:129]] + [
                    sA[:, k * 512:k * 512 + 129] for k in range(4)]
                for j in range(ST):
                    vj = vc_j(it, j)
                    for q in range(2, 8):
                        nc.tensor.matmul(
                            dsts[q - 2],
                            ex[:, j * S + q * 128:j * S + (q + 1) * 128],
                            vj,
                            start=(j == 0), stop=(j == ST - 1),
                        )
                for q in range(2, 4):
                    nc.vector.tensor_copy(outt[:, q * VCW:q * VCW + 132],
                                          tls2[q - 2][:, 0:132])
                for q in range(4, 8):
                    nc.scalar.copy(outt[:, q * VCW:q * VCW + 129],
                                   dsts[q - 2])
                nc.sync.dma_start(OC[h][:, 2 * VCW:], outt[:, 2 * VCW:])
                ins[h] = None
                exps[h] = None

            # Software-pipelined by one head: loads prefetch one head ahead
            # (deeper prefetch starves head 0's DMA bandwidth); ACT(exp) of
            # head h overlaps PE's PV of head h-1.
            emit_load(0)
            last_sA = None
            for h in range(HPC):
                if h + 1 < HPC:
                    emit_load(h + 1)
                emit_qk(h)
                if h == HPC - 1:
                    # the A tile of the last head, reused for chase groups
                    last_sA = psA_pool.tile([128, 2048], f32, tag="sc")
                if h >= 1:
                    emit_pv(h - 1)
            emit_pv_chase(HPC - 1, last_sA)

    nc.compile()
    return nc


def _get_nc():
    global _NC
    if _NC is None:
        _NC = _build_bass()
    return _NC


def kernel(Qx, Kx, Vx, Qy, Ky, Vy):
    global LAST_RESULTS
    bf = ml_dtypes.bfloat16
    Qx, Kx, Vx, Qy, Ky, Vy = (
        np.asarray(t, dtype=np.float32) for t in (Qx, Kx, Vx, Qy, Ky, Vy)
    )

    qx = Qx.reshape(HEADS, S, D)
    qy = Qy.reshape(HEADS, S, D)
    kx = Kx.reshape(HEADS, S, D)
    ky = Ky.reshape(HEADS, S, D)
    vx = Vx.reshape(HEADS, S, D)
    vy = Vy.reshape(HEADS, S, D)

    # Combined per-head input block: [head, p=128, INW] where
    #   [:, 0:S]        = QT * SCALE (x stream on partitions 0:64, y on 64:128)
    #   [:, S:2S]       = KT (same partition split)
    #   [:, 2S + j*VCW + c] = VC: kv position t = j*128+p; c in [Vx|Vy|1|pad]
    IN = np.zeros((HEADS, 128, INW), np.float32)
    IN[:, :D, 0:S] = qx.transpose(0, 2, 1) * SCALE
    IN[:, D:, 0:S] = qy.transpose(0, 2, 1) * SCALE
    IN[:, :D, S:2 * S] = kx.transpose(0, 2, 1)
    IN[:, D:, S:2 * S] = ky.transpose(0, 2, 1)
    vc = IN[:, :, 2 * S:].reshape(HEADS, 128, ST, VCW)
    vc[..., :D] = vx.reshape(HEADS, ST, 128, D).transpose(0, 2, 1, 3)
    vc[..., D:2 * D] = vy.reshape(HEADS, ST, 128, D).transpose(0, 2, 1, 3)
    vc[..., 2 * D] = 1.0

    in_maps = []
    for c in range(N_CORES):
        sl = slice(c * HPC, (c + 1) * HPC)
        in_maps.append({"inp": IN[sl].astype(bf)})

    from concourse.bass_utils import run_bass_kernel_spmd

    nc = _get_nc()
    res = run_bass_kernel_spmd(
        nc, in_maps, core_ids=list(range(N_CORES)), trace=TRACE, **TRACE_KW
    )
    LAST_RESULTS = res

    # oc: per core [HPC, p=128, ST*VCW]; per s-chunk q cols 0:64 out1_raw,
    # 64:128 out2_raw, col 128 sumexp -- softmax normalization here on host.
    oc = np.concatenate([r["oc"] for r in res.results], axis=0)
    oc = oc.reshape(HEADS, 128, ST, VCW).transpose(0, 2, 1, 3).reshape(B, H, S, VCW)
    z = oc[..., 2 * D:2 * D + 1]
    out1 = np.ascontiguousarray(oc[..., :D] / z)
    out2 = np.ascontiguousarray(oc[..., D:2 * D] / z)
    return out1, out2
